# revision 10
# baseline (speedup 1.0000x reference)
"""Trainium2 Bass kernel for the DeepFermi deconvolution GD problem.

Node-collapsed formulation: the per-pixel fermi sigmoid s1(tsh) is smooth on
the >=1 time-unit scale (k <= ~1), so the 512-point oversampled time grid is
replaced by Nc=16 interpolation nodes tshc; the piecewise-linear interp
matrix L is folded host-side into the convolution matrices:

    ML = M2 @ L, MVL = M2V @ L
    G  = ML^T ML,  Gv^T = ML^T MVL          (Nc x Nc Gram matrices)
    nw  = -(2/C_dc) ctc_dc @ ML             (per-pixel constants)
    nwv = -(2/C_dc) ctc_dc @ MVL

Per GD iteration, per pixel (s1c/sdc on the Nc nodes):
    w  = a2c*(G @ s1c)  + nw                (a2c = 2A/C_dc)
    wv = a2c*(Gv @ s1c) + nwv
    gA = w.s1c ; U = w.sdc ; V = wv.sdc
    gk = A(t0 U - V) ; gt0 = A k U          (+ prior & positivity terms)

Mapping: H sharded over 8 cores (16 rows each); tile = one H row = 128
pixels on partitions.  All 16 tiles are batched per iteration: tiles are
stacked along PE partitions for the transposed sigmoid (2 halves of 8
tiles x 16 nodes = 128), block-diagonal Gram matmuls produce y|yv for 8
tiles at once, and all DVE elementwise/reduce work runs at [128, 256-512]
free size.  eta columns are ordered A|t0|k so the k/t0 gradient + update +
kn-transpose (the only serial inter-iteration dependency) runs first;
the A-update and gA dot products happen in the shadow of the next
iteration's front end.
"""

import numpy as np

OSAMP = 8
MAX_ITER = 10
NEG_SHIFT = 2 * OSAMP
OTP = 5
C_SHARP = 500.0
LR = 0.1
T = 64
TOS = OSAMP * T  # 512
H = 128
W = 128
N_CORES = 8
ROWS_PER_CORE = H // N_CORES  # 16
TILES = ROWS_PER_CORE  # 16 tiles of 128 pixels per core
P = 128
NC = 16        # interpolation nodes
HGRP = 8       # tiles per half (stacked on PE partitions: 8*16 = 128)


# ---------------------------------------------------------------------------
# host-side math (iteration independent)
# ---------------------------------------------------------------------------

def _resize_mat(in_size, out_size):
    scale = out_size / in_size
    sample_f = (np.arange(out_size) + 0.5) / scale - 0.5
    x = np.abs(sample_f[None, :] - np.arange(in_size)[:, None])
    w = np.maximum(0.0, 1.0 - x)
    tot = w.sum(0, keepdims=True)
    w = np.where(np.abs(tot) > 1e-4, w / tot, 0.0)
    return w  # float64


def _sigmoid(x):
    with np.errstate(over="ignore"):
        return 1.0 / (1.0 + np.exp(-x))


def _interp_mat(tsh, tshc):
    Nc = len(tshc)
    Lm = np.zeros((len(tsh), Nc))
    j = np.searchsorted(tshc, tsh)
    for v, (x, jj) in enumerate(zip(tsh, j)):
        if jj <= 0:
            Lm[v, 0] = 1.0
        elif jj >= Nc:
            Lm[v, Nc - 1] = 1.0
        else:
            x0, x1 = tshc[jj - 1], tshc[jj]
            a = (x - x0) / (x1 - x0)
            Lm[v, jj - 1] = 1.0 - a
            Lm[v, jj] = a
    return Lm


def _preprocess(ctc, aif, time, eta_nn, lambda_reg):
    f64 = np.float64
    R = _resize_mat(T, TOS)
    aif0 = (aif.astype(f64) - aif.astype(f64)[..., :OTP].mean(-1, keepdims=True))
    ctc0 = (ctc.astype(f64) - ctc.astype(f64)[..., :OTP].mean(-1, keepdims=True))
    aif_os = (aif0 @ R)[0, 0, 0]                    # [512]
    t_os = time.astype(f64) @ R                     # [512]
    ctc_dc = (ctc0 @ R[:, ::OSAMP])[0]              # [H,W,64]
    C_dc = float((ctc_dc.astype(np.float32) ** 2).sum(dtype=np.float64))
    tsh = t_os - t_os[NEG_SHIFT]
    s2 = _sigmoid((C_SHARP * tsh).astype(np.float32).astype(f64))
    idx = NEG_SHIFT + 8 * np.arange(T)[:, None] - np.arange(TOS)[None, :]
    valid = (idx >= 0) & (idx <= TOS - 1)
    M = np.where(valid, aif_os[np.clip(idx, 0, TOS - 1)], 0.0) / OSAMP  # [64,512]
    M2 = M * s2[None, :]
    M2V = M2 * tsh[None, :]

    import ml_dtypes
    nidx = np.round(np.linspace(0, TOS - 1, NC)).astype(int)
    # bf16-representable node values so device arg matches host L exactly
    tshc = tsh[nidx].astype(ml_dtypes.bfloat16).astype(f64)
    Lm = _interp_mat(tsh, tshc)
    ML = M2 @ Lm                                    # [64, NC]
    MVL = M2V @ Lm
    G = ML.T @ ML                                   # [NC, NC] symmetric
    GvT = ML.T @ MVL                                # y_v = s1c @ GvT
    nw = -(2.0 / C_dc) * np.einsum('hwj,jc->hwc', ctc_dc, ML)    # [H,W,NC]
    nwv = -(2.0 / C_dc) * np.einsum('hwj,jc->hwc', ctc_dc, MVL)

    C_nn = (eta_nn.astype(f64) ** 2).sum(axis=(0, 2, 3))  # [3]
    sp_lam = np.logaddexp(0.0, float(lambda_reg.reshape(-1)[0]))
    creg = 2.0 * sp_lam / C_nn                      # [3]
    return tshc, G, GvT, nw, nwv, C_dc, creg


# ---------------------------------------------------------------------------
# bass module
# ---------------------------------------------------------------------------

_NC_CACHE = {}


def _build_nc():
    if "nc" in _NC_CACHE:
        return _NC_CACHE["nc"]

    import concourse.mybir as mybir
    import concourse.tile as tile
    from concourse import bacc

    dt = mybir.dt.float32
    bf = mybir.dt.bfloat16
    Alu = mybir.AluOpType
    Act = mybir.ActivationFunctionType
    X = mybir.AxisListType.X

    nc = bacc.Bacc("TRN2", target_bir_lowering=False, debug=False)

    # shared constants
    d_argwbd = nc.declare_dram_parameter("argwbd", [2 * TILES, TILES * NC],
                                         bf, isOutput=False)
    d_ggvd = nc.declare_dram_parameter("ggvd", [HGRP * NC, HGRP * 2 * NC], bf,
                                       isOutput=False)
    d_ident = nc.declare_dram_parameter("ident", [P, P], bf, isOutput=False)
    # per-core data
    d_nwfull = nc.declare_dram_parameter("nwfull", [P, TILES * 2 * NC], bf,
                                         isOutput=False)
    d_eta0 = nc.declare_dram_parameter("eta0", [P, 3 * TILES], dt, isOutput=False)
    d_cpl48 = nc.declare_dram_parameter("cpl48", [P, 3 * TILES], dt, isOutput=False)
    d_s48 = nc.declare_dram_parameter("s48", [P, 3 * TILES], dt, isOutput=False)
    d_toc = nc.declare_dram_parameter("toc", [P, TILES], dt, isOutput=False)
    d_out = nc.declare_dram_parameter("out", [P, 3 * TILES], dt, isOutput=True)

    NT2 = 2 * NC * TILES  # 512: w|wv free size for all tiles
    NT1 = NC * TILES      # 256

    with tile.TileContext(nc) as tc:
        with (
            tc.tile_pool(name="const", bufs=1) as cpool,
            tc.tile_pool(name="state", bufs=2) as spool,
            tc.tile_pool(name="work", bufs=2) as wpool,
            tc.tile_pool(name="small", bufs=2) as mpool,
            tc.tile_pool(name="ps_a", bufs=2, space="PSUM") as ps_a,
            tc.tile_pool(name="ps_b", bufs=2, space="PSUM") as ps_b,
            tc.tile_pool(name="ps_y", bufs=2, space="PSUM") as ps_y,
            tc.tile_pool(name="ps_k", bufs=1, space="PSUM") as ps_k,
        ):
            # ---- load constants (order matters: first-needed first) ----
            ident = cpool.tile([P, P], bf, tag="ident")
            nc.gpsimd.dma_start(ident[:], d_ident[:])
            eta_in = cpool.tile([P, 3 * TILES], dt, tag="eta_in")
            nc.gpsimd.dma_start(eta_in[:], d_eta0[:])
            toc = cpool.tile([P, TILES], dt, tag="toc")
            nc.gpsimd.dma_start(toc[:], d_toc[:])
            argwbd = cpool.tile([2 * TILES, TILES * NC], bf, tag="argwbd")
            nc.gpsimd.dma_start(argwbd[:], d_argwbd[:])
            ggvd = cpool.tile([HGRP * NC, HGRP * 2 * NC], bf, tag="ggvd")
            nc.gpsimd.dma_start(ggvd[:], d_ggvd[:])
            s48 = cpool.tile([P, 3 * TILES], dt, tag="s48")
            nc.gpsimd.dma_start(s48[:], d_s48[:])
            cpl48 = cpool.tile([P, 3 * TILES], dt, tag="cpl48")
            nc.gpsimd.dma_start(cpl48[:], d_cpl48[:])
            nwfull = cpool.tile([P, TILES * 2 * NC], bf, tag="nwfull")
            nc.gpsimd.dma_start(nwfull[:], d_nwfull[:])

            eta48 = eta_in

            def make_derived(e48):
                # eta columns: A | t0 | k ; kn columns: [k*t0 x16 | -k x16]
                eK = e48[:, 2 * TILES:3 * TILES]
                eT = e48[:, TILES:2 * TILES]
                kn = spool.tile([P, 2 * TILES], bf, tag="kn")
                nc.vector.tensor_tensor(kn[:, 0:TILES], eK, eT, Alu.mult)
                nc.vector.tensor_scalar_mul(kn[:, TILES:2 * TILES], eK, -1.0)
                knt_ps = ps_k.tile([2 * TILES, P], bf, tag="kntp")
                nc.tensor.transpose(knt_ps[:], kn[:], ident[:])
                knT = spool.tile([2 * TILES, P], bf, tag="knT")
                nc.scalar.copy(knT[:], knt_ps[:])
                return knT

            def make_a2c(e48):
                a2c = spool.tile([P, TILES], dt, tag="a2c")
                nc.vector.tensor_tensor(a2c[:], e48[:, 0:TILES], toc[:],
                                        Alu.mult)
                return a2c

            knT = make_derived(eta48)
            a2c = make_a2c(eta48)
            shadow = None  # deferred gA reduce + A-column update of prev iter

            for it in range(MAX_ITER):
                # ---- deferred shadow work from previous iteration ----
                if shadow is not None:
                    pBp, e48p, X48p = shadow
                    gA = mpool.tile([P, TILES], dt, tag="gA")
                    nc.vector.reduce_sum(
                        gA[:], pBp[:].rearrange("p (t c) -> p t c", t=TILES),
                        axis=X)
                    nc.vector.affine_then_add(e48p[:, 0:TILES], gA[:],
                                              X48p[:, 0:TILES], -LR, 0.0)
                    a2c = make_a2c(e48p)

                # ---- shadow: X48 = eta*s48 + m48 + cpl48 (GpSimd) ----
                m48 = mpool.tile([P, 3 * TILES], dt, tag="m48")
                nc.gpsimd.tensor_scalar(m48[:], eta48[:], 0.0, -2.0 * LR,
                                        Alu.min, Alu.mult)
                up48 = mpool.tile([P, 3 * TILES], dt, tag="up48")
                nc.gpsimd.tensor_tensor(up48[:], eta48[:], s48[:], Alu.mult)
                xb = mpool.tile([P, 3 * TILES], dt, tag="xb")
                nc.gpsimd.tensor_tensor(xb[:], up48[:], m48[:], Alu.add)
                X48 = mpool.tile([P, 3 * TILES], dt, tag="X48")
                nc.gpsimd.tensor_tensor(X48[:], xb[:], cpl48[:], Alu.add)

                # ---- pixel-major arg + sigmoid + sdc (off critical path) ----
                arg2 = ps_b.tile([P, NT1], dt, tag="arg2")
                nc.tensor.matmul(arg2[:], knT[:], argwbd[:], start=True,
                                 stop=True)
                s1c = wpool.tile([P, NT1], bf, tag="s1c")
                nc.scalar.activation(s1c[:], arg2[:], Act.Sigmoid)
                sdc = wpool.tile([P, NT1], bf, tag="sdc")
                sdacc = wpool.tile([P, 1], dt, tag="sdacc")
                nc.vector.affine_mul_reduce(sdc[:], sdacc[:], s1c[:], s1c[:],
                                            -1.0, 1.0)

                # ---- transposed sigmoid + Gram matmuls ----
                argT = ps_a.tile([P, 2 * P], dt, tag="argT")
                for h in range(2):
                    nc.tensor.matmul(argT[:, h * P:(h + 1) * P],
                                     argwbd[:, h * P:(h + 1) * P], knT[:],
                                     start=True, stop=True)
                s1cT = wpool.tile([P, 2 * P], bf, tag="s1cT")
                nc.scalar.activation(s1cT[:], argT[:], Act.Sigmoid)
                yps = ps_y.tile([P, NT2], dt, tag="yps")
                for h in range(2):
                    nc.tensor.matmul(yps[:, h * NT1:(h + 1) * NT1],
                                     s1cT[:, h * P:(h + 1) * P], ggvd[:],
                                     start=True, stop=True)

                # ---- w|wv = a2c*y + nw (PSUM read, broadcast a2c) ----
                a2cB = a2c[:].unsqueeze(2).broadcast_to([P, TILES, 2 * NC])
                w1 = wpool.tile([P, NT2], bf, tag="w1")
                w1v = w1[:].rearrange("p (t n) -> p t n", t=TILES)
                ypsv = yps[:].rearrange("p (t n) -> p t n", t=TILES)
                nc.vector.tensor_tensor(w1v, ypsv, a2cB, Alu.mult)
                wg = wpool.tile([P, NT2], bf, tag="wg")
                nc.vector.tensor_tensor(wg[:], w1[:], nwfull[:], Alu.add)

                # ---- products + segmented reduces ----
                w4 = wg[:].rearrange("p (t u c) -> p t u c", t=TILES, u=2,
                                     c=NC)
                sdc4 = sdc[:].rearrange("p (t c) -> p t c", t=TILES)\
                    .unsqueeze(2).broadcast_to([P, TILES, 2, NC])
                pA = wpool.tile([P, NT2], bf, tag="pA")
                pA4 = pA[:].rearrange("p (t u c) -> p t u c", t=TILES, u=2,
                                      c=NC)
                nc.vector.tensor_tensor(pA4, w4, sdc4, Alu.mult)
                UVm = mpool.tile([P, 2 * TILES], dt, tag="UVm")
                nc.vector.reduce_sum(
                    UVm[:].rearrange("p (t u) -> p t u", t=TILES), pA4, axis=X)

                # ---- critical path: k/t0 gradient, update, kn transpose ----
                eA = eta48[:, 0:TILES]
                eT = eta48[:, TILES:2 * TILES]
                eK = eta48[:, 2 * TILES:3 * TILES]
                p12 = mpool.tile([P, 2 * TILES], dt, tag="p12")
                eAB = eA.unsqueeze(2).broadcast_to([P, TILES, 2])
                nc.vector.tensor_tensor(
                    p12[:].rearrange("p (t u) -> p t u", t=TILES),
                    UVm[:].rearrange("p (t u) -> p t u", t=TILES),
                    eAB, Alu.mult)
                p1 = p12[:, 0:2 * TILES:2]
                p2 = p12[:, 1:2 * TILES:2]
                gkt = mpool.tile([P, 2 * TILES], dt, tag="gkt")
                # cols 0:16 = gt0 = k*p1 ; cols 16:32 = gk = t0*p1 - p2
                nc.vector.tensor_tensor(gkt[:, 0:TILES], p1, eK, Alu.mult)
                wk16 = mpool.tile([P, TILES], dt, tag="wk16")
                nc.vector.tensor_tensor(wk16[:], p1, eT, Alu.mult)
                nc.vector.tensor_tensor(gkt[:, TILES:2 * TILES], wk16[:], p2,
                                        Alu.subtract)
                eta48n = spool.tile([P, 3 * TILES], dt, tag="eta48")
                nc.vector.affine_then_add(eta48n[:, TILES:3 * TILES], gkt[:],
                                          X48[:, TILES:3 * TILES], -LR, 0.0)
                if it < MAX_ITER - 1:
                    knT = make_derived(eta48n)

                # ---- gA product on GpSimd; reduce + A-update deferred ----
                pB = wpool.tile([P, NT1], bf, tag="pB")
                pB3 = pB[:].rearrange("p (t c) -> p t c", t=TILES)
                s1c3 = s1c[:].rearrange("p (t c) -> p t c", t=TILES)
                nc.gpsimd.tensor_tensor(pB3, w4[:, :, 0, :], s1c3, Alu.mult)
                shadow = (pB, eta48n, X48)
                eta48 = eta48n

            # flush the last iteration's deferred A-column update
            pBp, e48p, X48p = shadow
            gA = mpool.tile([P, TILES], dt, tag="gA")
            nc.vector.reduce_sum(
                gA[:], pBp[:].rearrange("p (t c) -> p t c", t=TILES), axis=X)
            nc.vector.affine_then_add(e48p[:, 0:TILES], gA[:],
                                      X48p[:, 0:TILES], -LR, 0.0)
            nc.gpsimd.dma_start(d_out[:], eta48[:])

    nc.finalize()
    _NC_CACHE["nc"] = nc
    return nc


# ---------------------------------------------------------------------------
# input staging (eta column order: A | t0 | k)
# ---------------------------------------------------------------------------

def _make_in_maps(ctc, aif, time, eta_nn, lambda_reg):
    f32 = np.float32
    import ml_dtypes
    bf16 = ml_dtypes.bfloat16

    tshc, G, GvT, nw, nwv, C_dc, creg = _preprocess(
        ctc, aif, time, eta_nn, lambda_reg)

    toc_v = 2.0 / C_dc
    sA, sK, sT0 = (1.0 - LR * creg).astype(np.float64)

    # argwbd[k, 16*t + c]: row t -> 1.0, row 16+t -> tshc[c]
    # (matches kn layout [k*t0 x16 | -k x16])
    argwbd = np.zeros((2 * TILES, TILES * NC), bf16)
    tshcf = tshc.astype(f32)
    for t_ in range(TILES):
        argwbd[t_, t_ * NC:(t_ + 1) * NC] = 1.0
        argwbd[TILES + t_, t_ * NC:(t_ + 1) * NC] = tshcf
    # ggvd block-diag (8 tiles per half): [16,32] blocks of [G | Gv^T]
    ggvd = np.zeros((HGRP * NC, HGRP * 2 * NC), bf16)
    blk = np.concatenate([G, GvT], axis=1)          # [NC, 2*NC]
    for tau in range(HGRP):
        ggvd[tau * NC:(tau + 1) * NC, tau * 2 * NC:(tau + 1) * 2 * NC] = blk
    ident = np.eye(P, dtype=bf16)

    toc = np.full((P, TILES), toc_v, f32)
    s48 = np.zeros((P, 3 * TILES), f32)
    s48[:, 0:TILES] = sA          # A
    s48[:, TILES:2 * TILES] = sT0  # t0
    s48[:, 2 * TILES:] = sK        # k

    in_maps = []
    for m in range(N_CORES):
        rows = slice(m * ROWS_PER_CORE, (m + 1) * ROWS_PER_CORE)
        nwc = np.stack([nw[rows], nwv[rows]], axis=2)  # [16,128,2,NC]
        nwfull = np.ascontiguousarray(
            nwc.transpose(1, 0, 2, 3).reshape(P, TILES * 2 * NC)).astype(bf16)
        pr = eta_nn[0, :, rows, :].astype(np.float64)   # [3(A,k,t0), 16, 128]
        pr_atk = pr[[0, 2, 1]]                          # A | t0 | k
        eta0 = np.ascontiguousarray(
            pr_atk.transpose(2, 0, 1).reshape(P, 3 * TILES)).astype(f32)
        creg_atk = creg[[0, 2, 1]]
        cpl48 = np.zeros((P, 3 * TILES), f32)
        for c in range(3):
            cpl48[:, c * TILES:(c + 1) * TILES] = (
                LR * creg_atk[c] * pr_atk[c]).T
        in_maps.append({
            "argwbd": argwbd, "ggvd": ggvd, "ident": ident,
            "nwfull": nwfull, "eta0": eta0, "cpl48": cpl48, "s48": s48,
            "toc": toc,
        })
    return in_maps


def _emulate(in_maps):
    """Numpy replay of the device pipeline from staged arrays (debug aid)."""
    import ml_dtypes
    bf16 = ml_dtypes.bfloat16
    f32 = np.float32

    def bfq(x):
        return np.asarray(x, dtype=f32).astype(bf16).astype(f32)

    outs = []
    for mp in in_maps:
        argwbd = mp["argwbd"].astype(f32)
        ggvd = mp["ggvd"].astype(f32)
        nwfull = mp["nwfull"].astype(f32)
        eta48 = mp["eta0"].astype(f32).copy()
        cpl48 = mp["cpl48"]
        s48 = mp["s48"]
        toc = mp["toc"]
        for it in range(MAX_ITER):
            eA = eta48[:, 0:TILES]
            eT = eta48[:, TILES:2 * TILES]
            eK = eta48[:, 2 * TILES:]
            kn = np.zeros((P, 2 * TILES), f32)
            kn[:, 0:TILES] = bfq(eK * eT)
            kn[:, TILES:] = bfq(-eK)
            knT = kn.T  # [32, 128]
            a2c = eA * toc
            X48 = eta48 * s48 + np.minimum(eta48, 0.0) * (-2.0 * LR) + cpl48
            arg2 = knT.T @ argwbd          # [128, 256]
            s1c = bfq(_sigmoid(arg2))
            sdc = bfq(s1c * (1.0 - s1c))
            yps = np.zeros((P, 2 * NC * TILES), f32)
            for h in range(2):
                argT = argwbd[:, h * P:(h + 1) * P].T @ knT   # [128, 128]
                s1cT = bfq(_sigmoid(argT))
                yps[:, h * 2 * NC * HGRP:(h + 1) * 2 * NC * HGRP] = \
                    s1cT.T @ ggvd
            w1 = bfq(yps.reshape(P, TILES, 2 * NC)
                     * a2c[:, :, None]).reshape(P, -1)
            wg = bfq(w1 + nwfull)
            w4 = wg.reshape(P, TILES, 2, NC)
            pA = bfq(w4 * sdc.reshape(P, TILES, 1, NC))
            UV = pA.sum(-1)                 # [128, 16, 2]
            p1 = eA * UV[:, :, 0]
            p2 = eA * UV[:, :, 1]
            gt0 = p1 * eK
            gk = p1 * eT - p2
            pB = bfq(w4[:, :, 0, :] * s1c.reshape(P, TILES, NC))
            gA = pB.sum(-1)
            G48 = np.concatenate([gA, gt0, gk], axis=1)
            eta48 = X48 - LR * G48
        outs.append(eta48)
    out = np.zeros((1, 3, H, W), f32)
    for m, arr in enumerate(outs):
        rows = slice(m * ROWS_PER_CORE, (m + 1) * ROWS_PER_CORE)
        a3 = arr.reshape(P, 3, TILES)      # A | t0 | k
        out[0, 0, rows, :] = a3[:, 0, :].T
        out[0, 1, rows, :] = a3[:, 2, :].T
        out[0, 2, rows, :] = a3[:, 1, :].T
    return out


# ---------------------------------------------------------------------------
# public entry point
# ---------------------------------------------------------------------------

def kernel(ctc, aif, time, seg, eta_nn, lambda_reg):
    from concourse.bass_utils import run_bass_kernel_spmd

    ctc = np.asarray(ctc)
    aif = np.asarray(aif)
    time = np.asarray(time)
    eta_nn = np.asarray(eta_nn)
    lambda_reg = np.asarray(lambda_reg)

    in_maps = _make_in_maps(ctc, aif, time, eta_nn, lambda_reg)
    nc = _build_nc()
    res = run_bass_kernel_spmd(nc, in_maps, list(range(N_CORES)))

    out = np.zeros((1, 3, H, W), np.float32)
    for m in range(N_CORES):
        rows = slice(m * ROWS_PER_CORE, (m + 1) * ROWS_PER_CORE)
        arr = res.results[m]["out"]                  # [128, 48] A|t0|k
        a3 = arr.reshape(P, 3, TILES)
        out[0, 0, rows, :] = a3[:, 0, :].T
        out[0, 1, rows, :] = a3[:, 2, :].T
        out[0, 2, rows, :] = a3[:, 1, :].T
    return out


# revision 12
# speedup vs baseline: 1.2472x; 1.2472x over previous
"""Trainium2 Bass kernel for the DeepFermi deconvolution GD problem.

Node-collapsed formulation: the per-pixel fermi sigmoid s1(tsh) is smooth on
the >=1 time-unit scale (k <= ~1), so the 512-point oversampled time grid is
replaced by Nc=8 interpolation nodes tshc; the piecewise-linear interp
matrix L is folded host-side into the convolution matrices:

    ML = M2 @ L, MVL = M2V @ L
    G  = ML^T ML,  Gv^T = ML^T MVL          (Nc x Nc Gram matrices)
    nw  = -(2/C_dc) ctc_dc @ ML             (per-pixel constants)
    nwv = -(2/C_dc) ctc_dc @ MVL

Per GD iteration, per pixel (s1c/sdc on the Nc nodes):
    w  = a2c*(G @ s1c)  + nw                (a2c = 2A/C_dc)
    wv = a2c*(Gv @ s1c) + nwv
    gA = w.s1c ; U = w.sdc ; V = wv.sdc
    gk = A(t0 U - V) ; gt0 = A k U          (+ prior & positivity terms)

Mapping: H sharded over 8 cores (16 rows each); tile = one H row = 128
pixels on partitions.  With Nc=8, all 16 tiles stack exactly onto the 128
PE partitions (16 tiles x 8 nodes), so the transposed sigmoid is a single
[128,128] matmul + one activation, and the block-diagonal Gram matmul
produces y|yv for the whole core in one [128,256] matmul.  eta columns are
ordered A|t0|k; the k/t0 gradient chain (the serial inter-iteration
dependency through the kn transpose) is kept short, with the A-column
update and gA dot in its shadow.  Dummy matmuls keep the PE clock warm.
"""

import numpy as np

OSAMP = 8
MAX_ITER = 10
NEG_SHIFT = 2 * OSAMP
OTP = 5
C_SHARP = 500.0
LR = 0.1
T = 64
TOS = OSAMP * T  # 512
H = 128
W = 128
N_CORES = 8
ROWS_PER_CORE = H // N_CORES  # 16
TILES = ROWS_PER_CORE  # 16 tiles of 128 pixels per core
P = 128
NC = 8         # interpolation nodes; TILES*NC = 128 partitions exactly
N_DUMMY = 8    # PE-warming dummy matmuls per iteration


# ---------------------------------------------------------------------------
# host-side math (iteration independent)
# ---------------------------------------------------------------------------

def _resize_mat(in_size, out_size):
    scale = out_size / in_size
    sample_f = (np.arange(out_size) + 0.5) / scale - 0.5
    x = np.abs(sample_f[None, :] - np.arange(in_size)[:, None])
    w = np.maximum(0.0, 1.0 - x)
    tot = w.sum(0, keepdims=True)
    w = np.where(np.abs(tot) > 1e-4, w / tot, 0.0)
    return w  # float64


def _sigmoid(x):
    with np.errstate(over="ignore"):
        return 1.0 / (1.0 + np.exp(-x))


def _interp_mat(tsh, tshc):
    Nc = len(tshc)
    Lm = np.zeros((len(tsh), Nc))
    j = np.searchsorted(tshc, tsh)
    for v, (x, jj) in enumerate(zip(tsh, j)):
        if jj <= 0:
            Lm[v, 0] = 1.0
        elif jj >= Nc:
            Lm[v, Nc - 1] = 1.0
        else:
            x0, x1 = tshc[jj - 1], tshc[jj]
            a = (x - x0) / (x1 - x0)
            Lm[v, jj - 1] = 1.0 - a
            Lm[v, jj] = a
    return Lm


def _preprocess(ctc, aif, time, eta_nn, lambda_reg):
    f64 = np.float64
    R = _resize_mat(T, TOS)
    aif0 = (aif.astype(f64) - aif.astype(f64)[..., :OTP].mean(-1, keepdims=True))
    ctc0 = (ctc.astype(f64) - ctc.astype(f64)[..., :OTP].mean(-1, keepdims=True))
    aif_os = (aif0 @ R)[0, 0, 0]                    # [512]
    t_os = time.astype(f64) @ R                     # [512]
    ctc_dc = (ctc0 @ R[:, ::OSAMP])[0]              # [H,W,64]
    C_dc = float((ctc_dc.astype(np.float32) ** 2).sum(dtype=np.float64))
    tsh = t_os - t_os[NEG_SHIFT]
    s2 = _sigmoid((C_SHARP * tsh).astype(np.float32).astype(f64))
    idx = NEG_SHIFT + 8 * np.arange(T)[:, None] - np.arange(TOS)[None, :]
    valid = (idx >= 0) & (idx <= TOS - 1)
    M = np.where(valid, aif_os[np.clip(idx, 0, TOS - 1)], 0.0) / OSAMP  # [64,512]
    M2 = M * s2[None, :]
    M2V = M2 * tsh[None, :]

    import ml_dtypes
    nidx = np.round(np.linspace(0, TOS - 1, NC)).astype(int)
    # bf16-representable node values so device arg matches host L exactly
    tshc = tsh[nidx].astype(ml_dtypes.bfloat16).astype(f64)
    Lm = _interp_mat(tsh, tshc)
    ML = M2 @ Lm                                    # [64, NC]
    MVL = M2V @ Lm
    G = ML.T @ ML                                   # [NC, NC] symmetric
    GvT = ML.T @ MVL                                # y_v = s1c @ GvT
    nw = -(2.0 / C_dc) * np.einsum('hwj,jc->hwc', ctc_dc, ML)    # [H,W,NC]
    nwv = -(2.0 / C_dc) * np.einsum('hwj,jc->hwc', ctc_dc, MVL)

    C_nn = (eta_nn.astype(f64) ** 2).sum(axis=(0, 2, 3))  # [3]
    sp_lam = np.logaddexp(0.0, float(lambda_reg.reshape(-1)[0]))
    creg = 2.0 * sp_lam / C_nn                      # [3]
    return tshc, G, GvT, nw, nwv, C_dc, creg


# ---------------------------------------------------------------------------
# bass module
# ---------------------------------------------------------------------------

_NC_CACHE = {}

NT1 = NC * TILES      # 128: s1c/sdc free size
NT2 = 2 * NC * TILES  # 256: w|wv free size
FP_PACK = 3 * TILES * 3 + TILES   # eta0|cpl48|s48|toc packed cols (160)
BF_PACK = P + 2 * NC * TILES + TILES * 2 * NC  # ident|ggvd|nwfull (640)


def _build_nc():
    if "nc" in _NC_CACHE:
        return _NC_CACHE["nc"]

    import concourse.mybir as mybir
    import concourse.tile as tile
    from concourse import bacc

    dt = mybir.dt.float32
    bf = mybir.dt.bfloat16
    Alu = mybir.AluOpType
    Act = mybir.ActivationFunctionType
    X = mybir.AxisListType.X

    nc = bacc.Bacc("TRN2", target_bir_lowering=False, debug=False)

    d_argwbd = nc.declare_dram_parameter("argwbd", [2 * TILES, TILES * NC],
                                         bf, isOutput=False)
    d_bfpack = nc.declare_dram_parameter("bfpack", [P, BF_PACK], bf,
                                         isOutput=False)
    d_fppack = nc.declare_dram_parameter("fppack", [P, FP_PACK], dt,
                                         isOutput=False)
    d_out = nc.declare_dram_parameter("out", [P, 3 * TILES], dt, isOutput=True)

    with tile.TileContext(nc) as tc:
        with (
            tc.tile_pool(name="const", bufs=1) as cpool,
            tc.tile_pool(name="state", bufs=2) as spool,
            tc.tile_pool(name="work", bufs=2) as wpool,
            tc.tile_pool(name="small", bufs=2) as mpool,
            tc.tile_pool(name="ps_a", bufs=2, space="PSUM") as ps_a,
            tc.tile_pool(name="ps_b", bufs=2, space="PSUM") as ps_b,
            tc.tile_pool(name="ps_y", bufs=2, space="PSUM") as ps_y,
            tc.tile_pool(name="ps_k", bufs=1, space="PSUM") as ps_k,
            tc.tile_pool(name="ps_d", bufs=1, space="PSUM") as ps_d,
        ):
            # ---- load constants (3 packed DMAs) ----
            fppack = cpool.tile([P, FP_PACK], dt, tag="fppack")
            nc.gpsimd.dma_start(fppack[:], d_fppack[:])
            bfpack = cpool.tile([P, BF_PACK], bf, tag="bfpack")
            nc.gpsimd.dma_start(bfpack[:], d_bfpack[:])
            argwbd = cpool.tile([2 * TILES, TILES * NC], bf, tag="argwbd")
            nc.gpsimd.dma_start(argwbd[:], d_argwbd[:])

            eta_in = fppack[:, 0:3 * TILES]
            cpl48 = fppack[:, 3 * TILES:6 * TILES]
            s48 = fppack[:, 6 * TILES:9 * TILES]
            toc = fppack[:, 9 * TILES:10 * TILES]
            ident = bfpack[:, 0:P]
            ggvd = bfpack[:, P:P + NT2]
            nwfull = bfpack[:, P + NT2:P + 2 * NT2]

            eta48 = eta_in

            def make_derived(e48):
                # eta columns: A | t0 | k ; kn columns: [k*t0 x16 | -k x16]
                eK = e48[:, 2 * TILES:3 * TILES]
                eT = e48[:, TILES:2 * TILES]
                kn = spool.tile([P, 2 * TILES], bf, tag="kn")
                nc.vector.tensor_tensor(kn[:, 0:TILES], eK, eT, Alu.mult)
                nc.vector.tensor_scalar_mul(kn[:, TILES:2 * TILES], eK, -1.0)
                knt_ps = ps_k.tile([2 * TILES, P], bf, tag="kntp")
                nc.tensor.transpose(knt_ps[:], kn[:], ident)
                knT = spool.tile([2 * TILES, P], bf, tag="knT")
                nc.scalar.copy(knT[:], knt_ps[:])
                return knT

            def make_a2c(e48):
                a2c = spool.tile([P, TILES], dt, tag="a2c")
                nc.vector.tensor_tensor(a2c[:], e48[:, 0:TILES], toc,
                                        Alu.mult)
                return a2c

            knT = make_derived(eta48)
            a2c = make_a2c(eta48)

            for it in range(MAX_ITER):
                # ---- shadow: X48 = eta*s48 + m48 + cpl48 (GpSimd) ----
                m48 = mpool.tile([P, 3 * TILES], dt, tag="m48")
                nc.gpsimd.tensor_scalar(m48[:], eta48[:], 0.0, -2.0 * LR,
                                        Alu.min, Alu.mult)
                up48 = mpool.tile([P, 3 * TILES], dt, tag="up48")
                nc.gpsimd.tensor_tensor(up48[:], eta48[:], s48, Alu.mult)
                xb = mpool.tile([P, 3 * TILES], dt, tag="xb")
                nc.gpsimd.tensor_tensor(xb[:], up48[:], m48[:], Alu.add)
                X48 = mpool.tile([P, 3 * TILES], dt, tag="X48")
                nc.gpsimd.tensor_tensor(X48[:], xb[:], cpl48, Alu.add)

                # ---- args, sigmoids, Gram matmul ----
                arg2 = ps_b.tile([P, NT1], dt, tag="arg2")
                nc.tensor.matmul(arg2[:], knT[:], argwbd[:], start=True,
                                 stop=True)
                s1c = wpool.tile([P, NT1], bf, tag="s1c")
                nc.scalar.activation(s1c[:], arg2[:], Act.Sigmoid)
                sdc = wpool.tile([P, NT1], bf, tag="sdc")
                sdacc = wpool.tile([P, 1], dt, tag="sdacc")
                nc.vector.affine_mul_reduce(sdc[:], sdacc[:], s1c[:], s1c[:],
                                            -1.0, 1.0)

                argT = ps_a.tile([P, P], dt, tag="argT")
                nc.tensor.matmul(argT[:], argwbd[:], knT[:], start=True,
                                 stop=True)
                s1cT = wpool.tile([P, P], bf, tag="s1cT")
                nc.scalar.activation(s1cT[:], argT[:], Act.Sigmoid)
                yps = ps_y.tile([P, NT2], dt, tag="yps")
                nc.tensor.matmul(yps[:], s1cT[:], ggvd, start=True, stop=True)

                # ---- PE-warming dummies (no-op compute on scratch) ----
                scr = ps_d.tile([P, P], dt, tag="scr")
                for _ in range(N_DUMMY):
                    nc.tensor.matmul(scr[:], ident, ident, start=True,
                                     stop=True)

                # ---- w|wv = a2c*y + nw (PSUM read, broadcast a2c) ----
                a2cB = a2c[:].unsqueeze(2).broadcast_to([P, TILES, 2 * NC])
                w1 = wpool.tile([P, NT2], bf, tag="w1")
                nc.vector.tensor_tensor(
                    w1[:].rearrange("p (t n) -> p t n", t=TILES),
                    yps[:].rearrange("p (t n) -> p t n", t=TILES),
                    a2cB, Alu.mult)
                wg = wpool.tile([P, NT2], bf, tag="wg")
                nc.vector.tensor_tensor(wg[:], w1[:], nwfull, Alu.add)

                # ---- U|V products + segmented reduce ----
                w4 = wg[:].rearrange("p (t u c) -> p t u c", t=TILES, u=2,
                                     c=NC)
                sdc4 = sdc[:].rearrange("p (t c) -> p t c", t=TILES)\
                    .unsqueeze(2).broadcast_to([P, TILES, 2, NC])
                pA = wpool.tile([P, NT2], bf, tag="pA")
                pA4 = pA[:].rearrange("p (t u c) -> p t u c", t=TILES, u=2,
                                      c=NC)
                nc.vector.tensor_tensor(pA4, w4, sdc4, Alu.mult)
                UVm = mpool.tile([P, 2 * TILES], dt, tag="UVm")
                # out[t, u] -> col u*16 + t: U block then V block
                nc.vector.reduce_sum(
                    UVm[:].rearrange("p (u t) -> p t u", u=2), pA4, axis=X)

                # ---- critical path: k/t0 gradient, update, kn transpose ----
                # h2 = k*U ; h1 = t0*U - V ; (gt0|gk) = eA*(h2|h1)
                Um = UVm[:, 0:TILES]
                Vm = UVm[:, TILES:2 * TILES]
                eT = eta48[:, TILES:2 * TILES]
                eK = eta48[:, 2 * TILES:3 * TILES]
                h12 = mpool.tile([P, 2 * TILES], dt, tag="h12")
                nc.vector.tensor_tensor(h12[:, 0:TILES], Um, eK, Alu.mult)
                r1 = mpool.tile([P, TILES], dt, tag="r1")
                nc.vector.tensor_tensor(r1[:], Um, eT, Alu.mult)
                nc.vector.tensor_tensor(h12[:, TILES:2 * TILES], r1[:], Vm,
                                        Alu.subtract)
                gkt = mpool.tile([P, 2 * TILES], dt, tag="gkt")
                eAB = eta48[:, 0:TILES].unsqueeze(1).broadcast_to(
                    [P, 2, TILES])
                nc.vector.tensor_tensor(
                    gkt[:].rearrange("p (u t) -> p u t", u=2),
                    h12[:].rearrange("p (u t) -> p u t", u=2), eAB, Alu.mult)
                eta48n = spool.tile([P, 3 * TILES], dt, tag="eta48")
                nc.vector.affine_then_add(eta48n[:, TILES:3 * TILES], gkt[:],
                                          X48[:, TILES:3 * TILES], -LR, 0.0)
                if it < MAX_ITER - 1:
                    knT = make_derived(eta48n)

                # ---- shadow: gA dot, A update, a2c for next iter ----
                pB = wpool.tile([P, NT1], bf, tag="pB")
                pB3 = pB[:].rearrange("p (t c) -> p t c", t=TILES)
                s1c3 = s1c[:].rearrange("p (t c) -> p t c", t=TILES)
                nc.vector.tensor_tensor(pB3, w4[:, :, 0, :], s1c3, Alu.mult)
                gA = mpool.tile([P, TILES], dt, tag="gA")
                nc.vector.reduce_sum(gA[:], pB3, axis=X)
                nc.vector.affine_then_add(eta48n[:, 0:TILES], gA[:],
                                          X48[:, 0:TILES], -LR, 0.0)
                eta48 = eta48n
                if it < MAX_ITER - 1:
                    a2c = make_a2c(eta48)

            nc.gpsimd.dma_start(d_out[:], eta48[:])

    nc.finalize()
    _NC_CACHE["nc"] = nc
    return nc


# ---------------------------------------------------------------------------
# input staging (eta column order: A | t0 | k)
# ---------------------------------------------------------------------------

def _make_in_maps(ctc, aif, time, eta_nn, lambda_reg):
    f32 = np.float32
    import ml_dtypes
    bf16 = ml_dtypes.bfloat16

    tshc, G, GvT, nw, nwv, C_dc, creg = _preprocess(
        ctc, aif, time, eta_nn, lambda_reg)

    toc_v = 2.0 / C_dc
    sA, sK, sT0 = (1.0 - LR * creg).astype(np.float64)

    # argwbd[k, NC*t + c]: row t -> 1.0, row 16+t -> tshc[c]
    argwbd = np.zeros((2 * TILES, TILES * NC), bf16)
    tshcf = tshc.astype(f32)
    for t_ in range(TILES):
        argwbd[t_, t_ * NC:(t_ + 1) * NC] = 1.0
        argwbd[TILES + t_, t_ * NC:(t_ + 1) * NC] = tshcf
    # ggvd block-diag (16 tiles): [NC, 2*NC] blocks of [G | Gv^T]
    ggvd = np.zeros((TILES * NC, TILES * 2 * NC), bf16)
    blk = np.concatenate([G, GvT], axis=1)          # [NC, 2*NC]
    for tau in range(TILES):
        ggvd[tau * NC:(tau + 1) * NC, tau * 2 * NC:(tau + 1) * 2 * NC] = blk
    ident = np.eye(P, dtype=bf16)

    toc = np.full((P, TILES), toc_v, f32)
    s48 = np.zeros((P, 3 * TILES), f32)
    s48[:, 0:TILES] = sA          # A
    s48[:, TILES:2 * TILES] = sT0  # t0
    s48[:, 2 * TILES:] = sK        # k

    in_maps = []
    for m in range(N_CORES):
        rows = slice(m * ROWS_PER_CORE, (m + 1) * ROWS_PER_CORE)
        nwc = np.stack([nw[rows], nwv[rows]], axis=2)  # [16,128,2,NC]
        nwfull = np.ascontiguousarray(
            nwc.transpose(1, 0, 2, 3).reshape(P, TILES * 2 * NC)).astype(bf16)
        pr = eta_nn[0, :, rows, :].astype(np.float64)   # [3(A,k,t0), 16, 128]
        pr_atk = pr[[0, 2, 1]]                          # A | t0 | k
        eta0 = np.ascontiguousarray(
            pr_atk.transpose(2, 0, 1).reshape(P, 3 * TILES)).astype(f32)
        creg_atk = creg[[0, 2, 1]]
        cpl48 = np.zeros((P, 3 * TILES), f32)
        for c in range(3):
            cpl48[:, c * TILES:(c + 1) * TILES] = (
                LR * creg_atk[c] * pr_atk[c]).T
        fppack = np.concatenate([eta0, cpl48, s48, toc], axis=1)
        # ggvd is [128, 256] exactly (TILES*NC = 128)
        bfpack = np.concatenate([ident, ggvd.astype(bf16), nwfull], axis=1)
        in_maps.append({
            "argwbd": argwbd, "bfpack": bfpack, "fppack": fppack,
        })
    return in_maps


def _emulate(in_maps):
    """Numpy replay of the device pipeline from staged arrays (debug aid)."""
    import ml_dtypes
    bf16 = ml_dtypes.bfloat16
    f32 = np.float32

    def bfq(x):
        return np.asarray(x, dtype=f32).astype(bf16).astype(f32)

    outs = []
    for mp in in_maps:
        argwbd = mp["argwbd"].astype(f32)
        bfpack = mp["bfpack"].astype(f32)
        fppack = mp["fppack"]
        ggvd = bfpack[:, P:P + NT2]
        nwfull = bfpack[:, P + NT2:P + 2 * NT2]
        eta48 = fppack[:, 0:3 * TILES].astype(f32).copy()
        cpl48 = fppack[:, 3 * TILES:6 * TILES]
        s48 = fppack[:, 6 * TILES:9 * TILES]
        toc = fppack[:, 9 * TILES:10 * TILES]
        for it in range(MAX_ITER):
            eA = eta48[:, 0:TILES]
            eT = eta48[:, TILES:2 * TILES]
            eK = eta48[:, 2 * TILES:]
            kn = np.zeros((P, 2 * TILES), f32)
            kn[:, 0:TILES] = bfq(eK * eT)
            kn[:, TILES:] = bfq(-eK)
            knT = kn.T  # [32, 128]
            a2c = eA * toc
            X48 = eta48 * s48 + np.minimum(eta48, 0.0) * (-2.0 * LR) + cpl48
            arg2 = knT.T @ argwbd          # [128, 128]
            s1c = bfq(_sigmoid(arg2))
            sdc = bfq(s1c * (1.0 - s1c))
            argT = argwbd.T @ knT          # [128, 128]
            s1cT = bfq(_sigmoid(argT))
            yps = s1cT.T @ ggvd            # [128, 256]
            w1 = bfq(yps.reshape(P, TILES, 2 * NC)
                     * a2c[:, :, None]).reshape(P, -1)
            wg = bfq(w1 + nwfull)
            w4 = wg.reshape(P, TILES, 2, NC)
            pA = bfq(w4 * sdc.reshape(P, TILES, 1, NC))
            UV = pA.sum(-1)                 # [128, 16, 2]
            Um, Vm = UV[:, :, 0], UV[:, :, 1]
            h2 = Um * eK
            h1 = Um * eT - Vm
            gt0 = eA * h2
            gk = eA * h1
            pB = bfq(w4[:, :, 0, :] * s1c.reshape(P, TILES, NC))
            gA = pB.sum(-1)
            G48 = np.concatenate([gA, gt0, gk], axis=1)
            eta48 = X48 - LR * G48
        outs.append(eta48)
    out = np.zeros((1, 3, H, W), f32)
    for m, arr in enumerate(outs):
        rows = slice(m * ROWS_PER_CORE, (m + 1) * ROWS_PER_CORE)
        a3 = arr.reshape(P, 3, TILES)      # A | t0 | k
        out[0, 0, rows, :] = a3[:, 0, :].T
        out[0, 1, rows, :] = a3[:, 2, :].T
        out[0, 2, rows, :] = a3[:, 1, :].T
    return out


# ---------------------------------------------------------------------------
# public entry point
# ---------------------------------------------------------------------------

def kernel(ctc, aif, time, seg, eta_nn, lambda_reg):
    from concourse.bass_utils import run_bass_kernel_spmd

    ctc = np.asarray(ctc)
    aif = np.asarray(aif)
    time = np.asarray(time)
    eta_nn = np.asarray(eta_nn)
    lambda_reg = np.asarray(lambda_reg)

    in_maps = _make_in_maps(ctc, aif, time, eta_nn, lambda_reg)
    nc = _build_nc()
    res = run_bass_kernel_spmd(nc, in_maps, list(range(N_CORES)))

    out = np.zeros((1, 3, H, W), np.float32)
    for m in range(N_CORES):
        rows = slice(m * ROWS_PER_CORE, (m + 1) * ROWS_PER_CORE)
        arr = res.results[m]["out"]                  # [128, 48] A|t0|k
        a3 = arr.reshape(P, 3, TILES)
        out[0, 0, rows, :] = a3[:, 0, :].T
        out[0, 1, rows, :] = a3[:, 2, :].T
        out[0, 2, rows, :] = a3[:, 1, :].T
    return out


# revision 16
# speedup vs baseline: 1.2698x; 1.0182x over previous
"""Trainium2 Bass kernel for the DeepFermi deconvolution GD problem.

Node-collapsed formulation: the per-pixel fermi sigmoid s1(tsh) is smooth on
the >=1 time-unit scale (k <= ~1), so the 512-point oversampled time grid is
replaced by Nc=8 interpolation nodes tshc; the piecewise-linear interp
matrix L is folded host-side into the convolution matrices:

    ML = M2 @ L, MVL = M2V @ L
    G  = ML^T ML,  Gv^T = ML^T MVL          (Nc x Nc Gram matrices)
    nw  = -(2/C_dc) ctc_dc @ ML             (per-pixel constants)
    nwv = -(2/C_dc) ctc_dc @ MVL

Per GD iteration, per pixel (s1c/sdc on the Nc nodes):
    w  = a2c*(G @ s1c)  + nw                (a2c = 2A/C_dc)
    wv = a2c*(Gv @ s1c) + nwv
    gA = w.s1c ; U = w.sdc ; V = wv.sdc
    gk = A(t0 U - V) ; gt0 = A k U          (+ prior & positivity terms)

Mapping: H sharded over 8 cores (16 rows each); tile = one H row = 128
pixels on partitions.  With Nc=8, all 16 tiles stack exactly onto the 128
PE partitions (16 tiles x 8 nodes), so the transposed sigmoid is a single
[128,128] matmul + one activation, and the block-diagonal Gram matmul
produces y|yv for the whole core in one [128,256] matmul.  eta columns are
ordered A|t0|k; the k/t0 gradient chain (the serial inter-iteration
dependency through the kn transpose) is kept short, with the A-column
update and gA dot in its shadow.  Dummy matmuls keep the PE clock warm.
"""

import numpy as np

OSAMP = 8
MAX_ITER = 10
NEG_SHIFT = 2 * OSAMP
OTP = 5
C_SHARP = 500.0
LR = 0.1
T = 64
TOS = OSAMP * T  # 512
H = 128
W = 128
N_CORES = 8
ROWS_PER_CORE = H // N_CORES  # 16
TILES = ROWS_PER_CORE  # 16 tiles of 128 pixels per core
P = 128
NC = 8         # interpolation nodes; TILES*NC = 128 partitions exactly
N_DUMMY = 8    # PE-warming dummy matmuls per iteration


# ---------------------------------------------------------------------------
# host-side math (iteration independent)
# ---------------------------------------------------------------------------

def _resize_mat(in_size, out_size):
    scale = out_size / in_size
    sample_f = (np.arange(out_size) + 0.5) / scale - 0.5
    x = np.abs(sample_f[None, :] - np.arange(in_size)[:, None])
    w = np.maximum(0.0, 1.0 - x)
    tot = w.sum(0, keepdims=True)
    w = np.where(np.abs(tot) > 1e-4, w / tot, 0.0)
    return w  # float64


def _sigmoid(x):
    with np.errstate(over="ignore"):
        return 1.0 / (1.0 + np.exp(-x))


def _interp_mat(tsh, tshc):
    Nc = len(tshc)
    Lm = np.zeros((len(tsh), Nc))
    j = np.searchsorted(tshc, tsh)
    for v, (x, jj) in enumerate(zip(tsh, j)):
        if jj <= 0:
            Lm[v, 0] = 1.0
        elif jj >= Nc:
            Lm[v, Nc - 1] = 1.0
        else:
            x0, x1 = tshc[jj - 1], tshc[jj]
            a = (x - x0) / (x1 - x0)
            Lm[v, jj - 1] = 1.0 - a
            Lm[v, jj] = a
    return Lm


def _preprocess(ctc, aif, time, eta_nn, lambda_reg):
    f64 = np.float64
    R = _resize_mat(T, TOS)
    aif0 = (aif.astype(f64) - aif.astype(f64)[..., :OTP].mean(-1, keepdims=True))
    ctc0 = (ctc.astype(f64) - ctc.astype(f64)[..., :OTP].mean(-1, keepdims=True))
    aif_os = (aif0 @ R)[0, 0, 0]                    # [512]
    t_os = time.astype(f64) @ R                     # [512]
    ctc_dc = (ctc0 @ R[:, ::OSAMP])[0]              # [H,W,64]
    C_dc = float((ctc_dc.astype(np.float32) ** 2).sum(dtype=np.float64))
    tsh = t_os - t_os[NEG_SHIFT]
    s2 = _sigmoid((C_SHARP * tsh).astype(np.float32).astype(f64))
    idx = NEG_SHIFT + 8 * np.arange(T)[:, None] - np.arange(TOS)[None, :]
    valid = (idx >= 0) & (idx <= TOS - 1)
    M = np.where(valid, aif_os[np.clip(idx, 0, TOS - 1)], 0.0) / OSAMP  # [64,512]
    M2 = M * s2[None, :]
    M2V = M2 * tsh[None, :]

    import ml_dtypes
    nidx = np.round(np.linspace(0, TOS - 1, NC)).astype(int)
    # bf16-representable node values so device arg matches host L exactly
    tshc = tsh[nidx].astype(ml_dtypes.bfloat16).astype(f64)
    Lm = _interp_mat(tsh, tshc)
    ML = M2 @ Lm                                    # [64, NC]
    MVL = M2V @ Lm
    G = ML.T @ ML                                   # [NC, NC] symmetric
    GvT = ML.T @ MVL                                # y_v = s1c @ GvT
    nw = -(2.0 / C_dc) * np.einsum('hwj,jc->hwc', ctc_dc, ML)    # [H,W,NC]
    nwv = -(2.0 / C_dc) * np.einsum('hwj,jc->hwc', ctc_dc, MVL)

    C_nn = (eta_nn.astype(f64) ** 2).sum(axis=(0, 2, 3))  # [3]
    sp_lam = np.logaddexp(0.0, float(lambda_reg.reshape(-1)[0]))
    creg = 2.0 * sp_lam / C_nn                      # [3]
    return tshc, G, GvT, nw, nwv, C_dc, creg


# ---------------------------------------------------------------------------
# bass module
# ---------------------------------------------------------------------------

_NC_CACHE = {}

NT1 = NC * TILES      # 128: s1c/sdc free size
NT2 = 2 * NC * TILES  # 256: w|wv free size
FP_PACK = 3 * TILES * 3 + TILES   # eta0|cpl48|s48|toc packed cols (160)
BF_PACK = P + 2 * NC * TILES + TILES * 2 * NC  # ident|ggvd|nwfull (640)


def _build_nc():
    if "nc" in _NC_CACHE:
        return _NC_CACHE["nc"]

    import concourse.mybir as mybir
    import concourse.tile as tile
    from concourse import bacc

    dt = mybir.dt.float32
    bf = mybir.dt.bfloat16
    Alu = mybir.AluOpType
    Act = mybir.ActivationFunctionType
    X = mybir.AxisListType.X

    nc = bacc.Bacc("TRN2", target_bir_lowering=False, debug=False)

    d_argwbd = nc.declare_dram_parameter("argwbd", [2 * TILES, TILES * NC],
                                         bf, isOutput=False)
    d_bfpack = nc.declare_dram_parameter("bfpack", [P, BF_PACK], bf,
                                         isOutput=False)
    d_fppack = nc.declare_dram_parameter("fppack", [P, FP_PACK], dt,
                                         isOutput=False)
    d_out = nc.declare_dram_parameter("out", [P, 3 * TILES], dt, isOutput=True)

    with tile.TileContext(nc) as tc:
        with (
            tc.tile_pool(name="const", bufs=1) as cpool,
            tc.tile_pool(name="state", bufs=2) as spool,
            tc.tile_pool(name="work", bufs=2) as wpool,
            tc.tile_pool(name="small", bufs=2) as mpool,
            tc.tile_pool(name="ps_a", bufs=2, space="PSUM") as ps_a,
            tc.tile_pool(name="ps_b", bufs=2, space="PSUM") as ps_b,
            tc.tile_pool(name="ps_y", bufs=2, space="PSUM") as ps_y,
            tc.tile_pool(name="ps_k", bufs=1, space="PSUM") as ps_k,
            tc.tile_pool(name="ps_d", bufs=1, space="PSUM") as ps_d,
        ):
            # ---- load constants (3 packed DMAs) ----
            fppack = cpool.tile([P, FP_PACK], dt, tag="fppack")
            nc.gpsimd.dma_start(fppack[:], d_fppack[:])
            bfpack = cpool.tile([P, BF_PACK], bf, tag="bfpack")
            nc.gpsimd.dma_start(bfpack[:], d_bfpack[:])
            argwbd = cpool.tile([2 * TILES, TILES * NC], bf, tag="argwbd")
            nc.gpsimd.dma_start(argwbd[:], d_argwbd[:])

            eta_in = fppack[:, 0:3 * TILES]
            cpl48 = fppack[:, 3 * TILES:6 * TILES]
            s48 = fppack[:, 6 * TILES:9 * TILES]
            toc = fppack[:, 9 * TILES:10 * TILES]
            ident = bfpack[:, 0:P]
            ggvd = bfpack[:, P:P + NT2]
            nwfull = bfpack[:, P + NT2:P + 2 * NT2]

            eta48 = eta_in

            def make_derived(e48):
                # eta columns: A | t0 | k ; kn columns: [k*t0 x16 | k x16]
                # (the minus sign lives in argwbd's -tshc rows)
                eK = e48[:, 2 * TILES:3 * TILES]
                eT = e48[:, TILES:2 * TILES]
                kn = spool.tile([P, 2 * TILES], bf, tag="kn")
                nc.vector.tensor_tensor(kn[:, 0:TILES], eK, eT, Alu.mult)
                nc.vector.tensor_copy(kn[:, TILES:2 * TILES], eK)
                knt_ps = ps_k.tile([2 * TILES, P], bf, tag="kntp")
                nc.tensor.transpose(knt_ps[:], kn[:], ident)
                knT = spool.tile([2 * TILES, P], bf, tag="knT")
                nc.scalar.copy(knT[:], knt_ps[:])
                return knT

            def make_a2c(e48):
                a2c = spool.tile([P, TILES], dt, tag="a2c")
                nc.vector.tensor_tensor(a2c[:], e48[:, 0:TILES], toc,
                                        Alu.mult)
                return a2c

            knT = make_derived(eta48)
            a2c = make_a2c(eta48)

            for it in range(MAX_ITER):
                # ---- shadow: X48 = eta*s48 + m48 + cpl48 (GpSimd) ----
                m48 = mpool.tile([P, 3 * TILES], dt, tag="m48")
                nc.gpsimd.tensor_scalar(m48[:], eta48[:], 0.0, -2.0 * LR,
                                        Alu.min, Alu.mult)
                up48 = mpool.tile([P, 3 * TILES], dt, tag="up48")
                nc.gpsimd.tensor_tensor(up48[:], eta48[:], s48, Alu.mult)
                xb = mpool.tile([P, 3 * TILES], dt, tag="xb")
                nc.gpsimd.tensor_tensor(xb[:], up48[:], m48[:], Alu.add)
                X48 = mpool.tile([P, 3 * TILES], dt, tag="X48")
                nc.gpsimd.tensor_tensor(X48[:], xb[:], cpl48, Alu.add)

                # ---- args, sigmoids, Gram matmul ----
                arg2 = ps_b.tile([P, NT1], dt, tag="arg2")
                nc.tensor.matmul(arg2[:], knT[:], argwbd[:], start=True,
                                 stop=True)
                s1c = wpool.tile([P, NT1], bf, tag="s1c")
                nc.scalar.activation(s1c[:], arg2[:], Act.Sigmoid)
                sdc = wpool.tile([P, NT1], bf, tag="sdc")
                sdacc = wpool.tile([P, 1], dt, tag="sdacc")
                nc.vector.affine_mul_reduce(sdc[:], sdacc[:], s1c[:], s1c[:],
                                            -1.0, 1.0)

                argT = ps_a.tile([P, P], dt, tag="argT")
                nc.tensor.matmul(argT[:], argwbd[:], knT[:], start=True,
                                 stop=True)
                s1cT = wpool.tile([P, P], bf, tag="s1cT")
                nc.scalar.activation(s1cT[:], argT[:], Act.Sigmoid)
                yps = ps_y.tile([P, NT2], dt, tag="yps")
                nc.tensor.matmul(yps[:], s1cT[:], ggvd, start=True, stop=True)

                # ---- PE-warming dummies (knT dep pins them in this iter) ----
                scr = ps_d.tile([P, P], dt, tag="scr")
                for _ in range(N_DUMMY):
                    nc.tensor.matmul(scr[:], knT[:], ident[0:2 * TILES, :],
                                     start=True, stop=True)

                # ---- w|wv = a2c*y + nw (PSUM read, broadcast a2c) ----
                a2cB = a2c[:].unsqueeze(2).broadcast_to([P, TILES, 2 * NC])
                w1 = wpool.tile([P, NT2], bf, tag="w1")
                nc.vector.tensor_tensor(
                    w1[:].rearrange("p (t n) -> p t n", t=TILES),
                    yps[:].rearrange("p (t n) -> p t n", t=TILES),
                    a2cB, Alu.mult)
                wg = wpool.tile([P, NT2], bf, tag="wg")
                nc.vector.tensor_tensor(wg[:], w1[:], nwfull, Alu.add)

                # ---- U|V products + segmented reduce ----
                w4 = wg[:].rearrange("p (t u c) -> p t u c", t=TILES, u=2,
                                     c=NC)
                sdc4 = sdc[:].rearrange("p (t c) -> p t c", t=TILES)\
                    .unsqueeze(2).broadcast_to([P, TILES, 2, NC])
                pA = wpool.tile([P, NT2], bf, tag="pA")
                pA4 = pA[:].rearrange("p (t u c) -> p t u c", t=TILES, u=2,
                                      c=NC)
                nc.vector.tensor_tensor(pA4, w4, sdc4, Alu.mult)
                UVm = mpool.tile([P, 2 * TILES], dt, tag="UVm")
                # out[t, u] -> col u*16 + t: U block then V block
                nc.vector.reduce_sum(
                    UVm[:].rearrange("p (u t) -> p t u", u=2), pA4, axis=X)

                # ---- critical path: k/t0 gradient, update, kn transpose ----
                # h2 = k*U ; h1 = t0*U - V ; (gt0|gk) = eA*(h2|h1)
                Um = UVm[:, 0:TILES]
                Vm = UVm[:, TILES:2 * TILES]
                eT = eta48[:, TILES:2 * TILES]
                eK = eta48[:, 2 * TILES:3 * TILES]
                h12 = mpool.tile([P, 2 * TILES], dt, tag="h12")
                nc.vector.tensor_tensor(h12[:, 0:TILES], Um, eK, Alu.mult)
                r1 = mpool.tile([P, TILES], dt, tag="r1")
                nc.vector.tensor_tensor(r1[:], Um, eT, Alu.mult)
                nc.vector.tensor_tensor(h12[:, TILES:2 * TILES], r1[:], Vm,
                                        Alu.subtract)
                gkt = mpool.tile([P, 2 * TILES], dt, tag="gkt")
                eAB = eta48[:, 0:TILES].unsqueeze(1).broadcast_to(
                    [P, 2, TILES])
                nc.vector.tensor_tensor(
                    gkt[:].rearrange("p (u t) -> p u t", u=2),
                    h12[:].rearrange("p (u t) -> p u t", u=2), eAB, Alu.mult)
                eta48n = spool.tile([P, 3 * TILES], dt, tag="eta48")
                nc.vector.affine_then_add(eta48n[:, TILES:3 * TILES], gkt[:],
                                          X48[:, TILES:3 * TILES], -LR, 0.0)
                if it < MAX_ITER - 1:
                    knT = make_derived(eta48n)

                # ---- shadow: gA dot, A update, a2c for next iter ----
                pB = wpool.tile([P, NT1], bf, tag="pB")
                pB3 = pB[:].rearrange("p (t c) -> p t c", t=TILES)
                s1c3 = s1c[:].rearrange("p (t c) -> p t c", t=TILES)
                nc.vector.tensor_tensor(pB3, w4[:, :, 0, :], s1c3, Alu.mult)
                gA = mpool.tile([P, TILES], dt, tag="gA")
                nc.vector.reduce_sum(gA[:], pB3, axis=X)
                nc.vector.affine_then_add(eta48n[:, 0:TILES], gA[:],
                                          X48[:, 0:TILES], -LR, 0.0)
                eta48 = eta48n
                if it < MAX_ITER - 1:
                    a2c = make_a2c(eta48)

            nc.gpsimd.dma_start(d_out[:], eta48[:])

    nc.finalize()
    _NC_CACHE["nc"] = nc
    return nc


# ---------------------------------------------------------------------------
# input staging (eta column order: A | t0 | k)
# ---------------------------------------------------------------------------

def _make_in_maps(ctc, aif, time, eta_nn, lambda_reg):
    f32 = np.float32
    import ml_dtypes
    bf16 = ml_dtypes.bfloat16

    tshc, G, GvT, nw, nwv, C_dc, creg = _preprocess(
        ctc, aif, time, eta_nn, lambda_reg)

    toc_v = 2.0 / C_dc
    sA, sK, sT0 = (1.0 - LR * creg).astype(np.float64)

    # argwbd[k, NC*t + c]: row t -> 1.0, row 16+t -> -tshc[c]
    # (kn carries +k; the minus sign lives here)
    argwbd = np.zeros((2 * TILES, TILES * NC), bf16)
    tshcf = tshc.astype(f32)
    for t_ in range(TILES):
        argwbd[t_, t_ * NC:(t_ + 1) * NC] = 1.0
        argwbd[TILES + t_, t_ * NC:(t_ + 1) * NC] = -tshcf
    # ggvd block-diag (16 tiles): [NC, 2*NC] blocks of [G | Gv^T]
    ggvd = np.zeros((TILES * NC, TILES * 2 * NC), bf16)
    blk = np.concatenate([G, GvT], axis=1)          # [NC, 2*NC]
    for tau in range(TILES):
        ggvd[tau * NC:(tau + 1) * NC, tau * 2 * NC:(tau + 1) * 2 * NC] = blk
    ident = np.eye(P, dtype=bf16)

    toc = np.full((P, TILES), toc_v, f32)
    s48 = np.zeros((P, 3 * TILES), f32)
    s48[:, 0:TILES] = sA          # A
    s48[:, TILES:2 * TILES] = sT0  # t0
    s48[:, 2 * TILES:] = sK        # k

    in_maps = []
    for m in range(N_CORES):
        rows = slice(m * ROWS_PER_CORE, (m + 1) * ROWS_PER_CORE)
        nwc = np.stack([nw[rows], nwv[rows]], axis=2)  # [16,128,2,NC]
        nwfull = np.ascontiguousarray(
            nwc.transpose(1, 0, 2, 3).reshape(P, TILES * 2 * NC)).astype(bf16)
        pr = eta_nn[0, :, rows, :].astype(np.float64)   # [3(A,k,t0), 16, 128]
        pr_atk = pr[[0, 2, 1]]                          # A | t0 | k
        eta0 = np.ascontiguousarray(
            pr_atk.transpose(2, 0, 1).reshape(P, 3 * TILES)).astype(f32)
        creg_atk = creg[[0, 2, 1]]
        cpl48 = np.zeros((P, 3 * TILES), f32)
        for c in range(3):
            cpl48[:, c * TILES:(c + 1) * TILES] = (
                LR * creg_atk[c] * pr_atk[c]).T
        fppack = np.concatenate([eta0, cpl48, s48, toc], axis=1)
        # ggvd is [128, 256] exactly (TILES*NC = 128)
        bfpack = np.concatenate([ident, ggvd.astype(bf16), nwfull], axis=1)
        in_maps.append({
            "argwbd": argwbd, "bfpack": bfpack, "fppack": fppack,
        })
    return in_maps


def _emulate(in_maps):
    """Numpy replay of the device pipeline from staged arrays (debug aid)."""
    import ml_dtypes
    bf16 = ml_dtypes.bfloat16
    f32 = np.float32

    def bfq(x):
        return np.asarray(x, dtype=f32).astype(bf16).astype(f32)

    outs = []
    for mp in in_maps:
        argwbd = mp["argwbd"].astype(f32)
        bfpack = mp["bfpack"].astype(f32)
        fppack = mp["fppack"]
        ggvd = bfpack[:, P:P + NT2]
        nwfull = bfpack[:, P + NT2:P + 2 * NT2]
        eta48 = fppack[:, 0:3 * TILES].astype(f32).copy()
        cpl48 = fppack[:, 3 * TILES:6 * TILES]
        s48 = fppack[:, 6 * TILES:9 * TILES]
        toc = fppack[:, 9 * TILES:10 * TILES]
        for it in range(MAX_ITER):
            eA = eta48[:, 0:TILES]
            eT = eta48[:, TILES:2 * TILES]
            eK = eta48[:, 2 * TILES:]
            kn = np.zeros((P, 2 * TILES), f32)
            kn[:, 0:TILES] = bfq(eK * eT)
            kn[:, TILES:] = bfq(eK)
            knT = kn.T  # [32, 128]
            a2c = eA * toc
            X48 = eta48 * s48 + np.minimum(eta48, 0.0) * (-2.0 * LR) + cpl48
            arg2 = knT.T @ argwbd          # [128, 128]
            s1c = bfq(_sigmoid(arg2))
            sdc = bfq(s1c * (1.0 - s1c))
            argT = argwbd.T @ knT          # [128, 128]
            s1cT = bfq(_sigmoid(argT))
            yps = s1cT.T @ ggvd            # [128, 256]
            w1 = bfq(yps.reshape(P, TILES, 2 * NC)
                     * a2c[:, :, None]).reshape(P, -1)
            wg = bfq(w1 + nwfull)
            w4 = wg.reshape(P, TILES, 2, NC)
            pA = bfq(w4 * sdc.reshape(P, TILES, 1, NC))
            UV = pA.sum(-1)                 # [128, 16, 2]
            Um, Vm = UV[:, :, 0], UV[:, :, 1]
            h2 = Um * eK
            h1 = Um * eT - Vm
            gt0 = eA * h2
            gk = eA * h1
            pB = bfq(w4[:, :, 0, :] * s1c.reshape(P, TILES, NC))
            gA = pB.sum(-1)
            G48 = np.concatenate([gA, gt0, gk], axis=1)
            eta48 = X48 - LR * G48
        outs.append(eta48)
    out = np.zeros((1, 3, H, W), f32)
    for m, arr in enumerate(outs):
        rows = slice(m * ROWS_PER_CORE, (m + 1) * ROWS_PER_CORE)
        a3 = arr.reshape(P, 3, TILES)      # A | t0 | k
        out[0, 0, rows, :] = a3[:, 0, :].T
        out[0, 1, rows, :] = a3[:, 2, :].T
        out[0, 2, rows, :] = a3[:, 1, :].T
    return out


# ---------------------------------------------------------------------------
# public entry point
# ---------------------------------------------------------------------------

def kernel(ctc, aif, time, seg, eta_nn, lambda_reg):
    from concourse.bass_utils import run_bass_kernel_spmd

    ctc = np.asarray(ctc)
    aif = np.asarray(aif)
    time = np.asarray(time)
    eta_nn = np.asarray(eta_nn)
    lambda_reg = np.asarray(lambda_reg)

    in_maps = _make_in_maps(ctc, aif, time, eta_nn, lambda_reg)
    nc = _build_nc()
    res = run_bass_kernel_spmd(nc, in_maps, list(range(N_CORES)))

    out = np.zeros((1, 3, H, W), np.float32)
    for m in range(N_CORES):
        rows = slice(m * ROWS_PER_CORE, (m + 1) * ROWS_PER_CORE)
        arr = res.results[m]["out"]                  # [128, 48] A|t0|k
        a3 = arr.reshape(P, 3, TILES)
        out[0, 0, rows, :] = a3[:, 0, :].T
        out[0, 1, rows, :] = a3[:, 2, :].T
        out[0, 2, rows, :] = a3[:, 1, :].T
    return out


# revision 17
# speedup vs baseline: 1.3938x; 1.0977x over previous
"""Trainium2 Bass kernel for the DeepFermi deconvolution GD problem.

Node-collapsed formulation: the per-pixel fermi sigmoid s1(tsh) is smooth on
the >=1 time-unit scale (k <= ~1), so the 512-point oversampled time grid is
replaced by Nc=8 interpolation nodes tshc; the piecewise-linear interp
matrix L is folded host-side into the convolution matrices:

    ML = M2 @ L, MVL = M2V @ L
    G  = ML^T ML,  Gv^T = ML^T MVL          (Nc x Nc Gram matrices)
    nw  = -(2/C_dc) ctc_dc @ ML             (per-pixel constants)
    nwv = -(2/C_dc) ctc_dc @ MVL

Per GD iteration, per pixel (s1c/sdc on the Nc nodes):
    w  = a2c*(G @ s1c)  + nw                (a2c = 2A/C_dc)
    wv = a2c*(Gv @ s1c) + nwv
    gA = w.s1c ; U = w.sdc ; V = wv.sdc
    gk = A(t0 U - V) ; gt0 = A k U          (+ prior & positivity terms)

Mapping: H sharded over 8 cores (16 rows each); tile = one H row = 128
pixels on partitions.  With Nc=8, all 16 tiles stack exactly onto the 128
PE partitions (16 tiles x 8 nodes), so the transposed sigmoid is a single
[128,128] matmul + one activation, and the block-diagonal Gram matmul
produces y|yv for the whole core in one [128,256] matmul.  eta columns are
ordered A|t0|k; the k/t0 gradient chain (the serial inter-iteration
dependency through the kn transpose) is kept short, with the A-column
update and gA dot in its shadow.  Dummy matmuls keep the PE clock warm.
"""

import numpy as np

OSAMP = 8
MAX_ITER = 10
NEG_SHIFT = 2 * OSAMP
OTP = 5
C_SHARP = 500.0
LR = 0.1
T = 64
TOS = OSAMP * T  # 512
H = 128
W = 128
N_CORES = 8
ROWS_PER_CORE = H // N_CORES  # 16
TILES = ROWS_PER_CORE  # 16 tiles of 128 pixels per core
P = 128
NC = 8         # interpolation nodes; TILES*NC = 128 partitions exactly
N_DUMMY = 8    # PE-warming dummy matmuls per iteration


# ---------------------------------------------------------------------------
# host-side math (iteration independent)
# ---------------------------------------------------------------------------

def _resize_mat(in_size, out_size):
    scale = out_size / in_size
    sample_f = (np.arange(out_size) + 0.5) / scale - 0.5
    x = np.abs(sample_f[None, :] - np.arange(in_size)[:, None])
    w = np.maximum(0.0, 1.0 - x)
    tot = w.sum(0, keepdims=True)
    w = np.where(np.abs(tot) > 1e-4, w / tot, 0.0)
    return w  # float64


def _sigmoid(x):
    with np.errstate(over="ignore"):
        return 1.0 / (1.0 + np.exp(-x))


def _interp_mat(tsh, tshc):
    Nc = len(tshc)
    Lm = np.zeros((len(tsh), Nc))
    j = np.searchsorted(tshc, tsh)
    for v, (x, jj) in enumerate(zip(tsh, j)):
        if jj <= 0:
            Lm[v, 0] = 1.0
        elif jj >= Nc:
            Lm[v, Nc - 1] = 1.0
        else:
            x0, x1 = tshc[jj - 1], tshc[jj]
            a = (x - x0) / (x1 - x0)
            Lm[v, jj - 1] = 1.0 - a
            Lm[v, jj] = a
    return Lm


def _preprocess(ctc, aif, time, eta_nn, lambda_reg):
    f64 = np.float64
    R = _resize_mat(T, TOS)
    aif0 = (aif.astype(f64) - aif.astype(f64)[..., :OTP].mean(-1, keepdims=True))
    ctc0 = (ctc.astype(f64) - ctc.astype(f64)[..., :OTP].mean(-1, keepdims=True))
    aif_os = (aif0 @ R)[0, 0, 0]                    # [512]
    t_os = time.astype(f64) @ R                     # [512]
    ctc_dc = (ctc0 @ R[:, ::OSAMP])[0]              # [H,W,64]
    C_dc = float((ctc_dc.astype(np.float32) ** 2).sum(dtype=np.float64))
    tsh = t_os - t_os[NEG_SHIFT]
    s2 = _sigmoid((C_SHARP * tsh).astype(np.float32).astype(f64))
    idx = NEG_SHIFT + 8 * np.arange(T)[:, None] - np.arange(TOS)[None, :]
    valid = (idx >= 0) & (idx <= TOS - 1)
    M = np.where(valid, aif_os[np.clip(idx, 0, TOS - 1)], 0.0) / OSAMP  # [64,512]
    M2 = M * s2[None, :]
    M2V = M2 * tsh[None, :]

    import ml_dtypes
    nidx = np.round(np.linspace(0, TOS - 1, NC)).astype(int)
    # bf16-representable node values so device arg matches host L exactly
    tshc = tsh[nidx].astype(ml_dtypes.bfloat16).astype(f64)
    Lm = _interp_mat(tsh, tshc)
    ML = M2 @ Lm                                    # [64, NC]
    MVL = M2V @ Lm
    G = ML.T @ ML                                   # [NC, NC] symmetric
    GvT = ML.T @ MVL                                # y_v = s1c @ GvT
    nw = -(2.0 / C_dc) * np.einsum('hwj,jc->hwc', ctc_dc, ML)    # [H,W,NC]
    nwv = -(2.0 / C_dc) * np.einsum('hwj,jc->hwc', ctc_dc, MVL)

    C_nn = (eta_nn.astype(f64) ** 2).sum(axis=(0, 2, 3))  # [3]
    sp_lam = np.logaddexp(0.0, float(lambda_reg.reshape(-1)[0]))
    creg = 2.0 * sp_lam / C_nn                      # [3]
    return tshc, G, GvT, nw, nwv, C_dc, creg


# ---------------------------------------------------------------------------
# bass module
# ---------------------------------------------------------------------------

_NC_CACHE = {}

NT1 = NC * TILES      # 128: s1c/sdc free size
NT2 = 2 * NC * TILES  # 256: w|wv free size
FP_PACK = 3 * TILES * 3 + 2 * TILES + P  # eta0|cpl48|s48|toc|ones16|identf
BF_PACK = 2 * NC * TILES + TILES * 2 * NC  # ggvd|nwfull (512)


def _build_nc():
    if "nc" in _NC_CACHE:
        return _NC_CACHE["nc"]

    import concourse.mybir as mybir
    import concourse.tile as tile
    from concourse import bacc

    dt = mybir.dt.float32
    bf = mybir.dt.bfloat16
    Alu = mybir.AluOpType
    Act = mybir.ActivationFunctionType
    X = mybir.AxisListType.X

    nc = bacc.Bacc("TRN2", target_bir_lowering=False, debug=False)

    d_argwbd = nc.declare_dram_parameter("argwbd", [2 * TILES, TILES * NC],
                                         bf, isOutput=False)
    d_bfpack = nc.declare_dram_parameter("bfpack", [P, BF_PACK], bf,
                                         isOutput=False)
    d_fppack = nc.declare_dram_parameter("fppack", [P, FP_PACK], dt,
                                         isOutput=False)
    d_out = nc.declare_dram_parameter("out", [P, 3 * TILES], dt, isOutput=True)

    with tile.TileContext(nc) as tc:
        with (
            tc.tile_pool(name="const", bufs=1) as cpool,
            tc.tile_pool(name="state", bufs=2) as spool,
            tc.tile_pool(name="work", bufs=2) as wpool,
            tc.tile_pool(name="small", bufs=2) as mpool,
            tc.tile_pool(name="ps_a", bufs=2, space="PSUM") as ps_a,
            tc.tile_pool(name="ps_b", bufs=2, space="PSUM") as ps_b,
            tc.tile_pool(name="ps_y", bufs=2, space="PSUM") as ps_y,
            tc.tile_pool(name="ps_k", bufs=1, space="PSUM") as ps_k,
            tc.tile_pool(name="ps_d", bufs=1, space="PSUM") as ps_d,
        ):
            # ---- load constants (3 packed DMAs) ----
            fppack = cpool.tile([P, FP_PACK], dt, tag="fppack")
            nc.gpsimd.dma_start(fppack[:], d_fppack[:])
            bfpack = cpool.tile([P, BF_PACK], bf, tag="bfpack")
            nc.gpsimd.dma_start(bfpack[:], d_bfpack[:])
            argwbd = cpool.tile([2 * TILES, TILES * NC], bf, tag="argwbd")
            nc.gpsimd.dma_start(argwbd[:], d_argwbd[:])

            eta_in = fppack[:, 0:3 * TILES]
            cpl48 = fppack[:, 3 * TILES:6 * TILES]
            s48 = fppack[:, 6 * TILES:9 * TILES]
            toc = fppack[:, 9 * TILES:10 * TILES]
            ones16 = fppack[:, 10 * TILES:11 * TILES]
            identf = fppack[:, 11 * TILES:11 * TILES + P]
            ggvd = bfpack[:, 0:NT2]
            nwfull = bfpack[:, NT2:2 * NT2]

            eta48 = eta_in

            def make_derived(e48):
                # eta columns: A | t0 | k ; kn columns: [k*t0 x16 | k x16]
                # (the minus sign lives in argwbd's -tshc rows)
                eK = e48[:, 2 * TILES:3 * TILES]
                eT = e48[:, TILES:2 * TILES]
                kn = spool.tile([P, 2 * TILES], dt, tag="kn")
                nc.vector.tensor_tensor(kn[:, 0:TILES], eK, eT, Alu.mult)
                nc.vector.tensor_tensor(kn[:, TILES:2 * TILES], eK, ones16,
                                        Alu.mult)
                knt_ps = ps_k.tile([2 * TILES, P], dt, tag="kntp")
                nc.tensor.transpose(knt_ps[:], kn[:], identf)
                knT = spool.tile([2 * TILES, P], bf, tag="knT")
                nc.scalar.copy(knT[:], knt_ps[:])
                return knT

            def make_a2c(e48):
                a2c = spool.tile([P, TILES], dt, tag="a2c")
                nc.vector.tensor_tensor(a2c[:], e48[:, 0:TILES], toc,
                                        Alu.mult)
                return a2c

            knT = make_derived(eta48)
            a2c = make_a2c(eta48)

            for it in range(MAX_ITER):
                # ---- shadow: X48 = eta*s48 + m48 + cpl48 (GpSimd) ----
                m48 = mpool.tile([P, 3 * TILES], dt, tag="m48")
                nc.gpsimd.tensor_scalar(m48[:], eta48[:], 0.0, -2.0 * LR,
                                        Alu.min, Alu.mult)
                up48 = mpool.tile([P, 3 * TILES], dt, tag="up48")
                nc.gpsimd.tensor_tensor(up48[:], eta48[:], s48, Alu.mult)
                xb = mpool.tile([P, 3 * TILES], dt, tag="xb")
                nc.gpsimd.tensor_tensor(xb[:], up48[:], m48[:], Alu.add)
                X48 = mpool.tile([P, 3 * TILES], dt, tag="X48")
                nc.gpsimd.tensor_tensor(X48[:], xb[:], cpl48, Alu.add)

                # ---- args, sigmoids, Gram matmul ----
                arg2 = ps_b.tile([P, NT1], dt, tag="arg2")
                nc.tensor.matmul(arg2[:], knT[:], argwbd[:], start=True,
                                 stop=True)
                s1c = wpool.tile([P, NT1], bf, tag="s1c")
                nc.scalar.activation(s1c[:], arg2[:], Act.Sigmoid)
                sdc = wpool.tile([P, NT1], bf, tag="sdc")
                sdacc = wpool.tile([P, 1], dt, tag="sdacc")
                nc.vector.affine_mul_reduce(sdc[:], sdacc[:], s1c[:], s1c[:],
                                            -1.0, 1.0)

                argT = ps_a.tile([P, P], dt, tag="argT")
                nc.tensor.matmul(argT[:], argwbd[:], knT[:], start=True,
                                 stop=True)
                s1cT = wpool.tile([P, P], bf, tag="s1cT")
                nc.scalar.activation(s1cT[:], argT[:], Act.Sigmoid)
                yps = ps_y.tile([P, NT2], dt, tag="yps")
                nc.tensor.matmul(yps[:], s1cT[:], ggvd, start=True, stop=True)

                # ---- PE-warming dummies (s1cT dep -> fill the DVE window) ----
                scr = ps_d.tile([P, P], dt, tag="scr")
                for _ in range(N_DUMMY):
                    nc.tensor.matmul(scr[:], s1cT[:], ggvd[:, 0:P],
                                     start=True, stop=True)

                # ---- w|wv = a2c*y + nw (PSUM read, broadcast a2c) ----
                a2cB = a2c[:].unsqueeze(2).broadcast_to([P, TILES, 2 * NC])
                w1 = wpool.tile([P, NT2], bf, tag="w1")
                nc.vector.tensor_tensor(
                    w1[:].rearrange("p (t n) -> p t n", t=TILES),
                    yps[:].rearrange("p (t n) -> p t n", t=TILES),
                    a2cB, Alu.mult)
                wg = wpool.tile([P, NT2], bf, tag="wg")
                nc.vector.tensor_tensor(wg[:], w1[:], nwfull, Alu.add)

                # ---- U|V products + segmented reduce ----
                w4 = wg[:].rearrange("p (t u c) -> p t u c", t=TILES, u=2,
                                     c=NC)
                sdc4 = sdc[:].rearrange("p (t c) -> p t c", t=TILES)\
                    .unsqueeze(2).broadcast_to([P, TILES, 2, NC])
                pA = wpool.tile([P, NT2], bf, tag="pA")
                pA4 = pA[:].rearrange("p (t u c) -> p t u c", t=TILES, u=2,
                                      c=NC)
                nc.vector.tensor_tensor(pA4, w4, sdc4, Alu.mult)
                UVm = mpool.tile([P, 2 * TILES], dt, tag="UVm")
                # out[t, u] -> col u*16 + t: U block then V block
                nc.vector.reduce_sum(
                    UVm[:].rearrange("p (u t) -> p t u", u=2), pA4, axis=X)

                # ---- critical path: k/t0 gradient, update, kn transpose ----
                # h2 = k*U ; h1 = t0*U - V ; (gt0|gk) = eA*(h2|h1)
                Um = UVm[:, 0:TILES]
                Vm = UVm[:, TILES:2 * TILES]
                eT = eta48[:, TILES:2 * TILES]
                eK = eta48[:, 2 * TILES:3 * TILES]
                h12 = mpool.tile([P, 2 * TILES], dt, tag="h12")
                nc.vector.tensor_tensor(h12[:, 0:TILES], Um, eK, Alu.mult)
                r1 = mpool.tile([P, TILES], dt, tag="r1")
                nc.vector.tensor_tensor(r1[:], Um, eT, Alu.mult)
                nc.vector.tensor_tensor(h12[:, TILES:2 * TILES], r1[:], Vm,
                                        Alu.subtract)
                gkt = mpool.tile([P, 2 * TILES], dt, tag="gkt")
                eAB = eta48[:, 0:TILES].unsqueeze(1).broadcast_to(
                    [P, 2, TILES])
                nc.vector.tensor_tensor(
                    gkt[:].rearrange("p (u t) -> p u t", u=2),
                    h12[:].rearrange("p (u t) -> p u t", u=2), eAB, Alu.mult)
                eta48n = spool.tile([P, 3 * TILES], dt, tag="eta48")
                nc.vector.affine_then_add(eta48n[:, TILES:3 * TILES], gkt[:],
                                          X48[:, TILES:3 * TILES], -LR, 0.0)
                if it < MAX_ITER - 1:
                    knT = make_derived(eta48n)

                # ---- shadow: gA dot, A update, a2c for next iter ----
                pB = wpool.tile([P, NT1], bf, tag="pB")
                pB3 = pB[:].rearrange("p (t c) -> p t c", t=TILES)
                s1c3 = s1c[:].rearrange("p (t c) -> p t c", t=TILES)
                nc.vector.tensor_tensor(pB3, w4[:, :, 0, :], s1c3, Alu.mult)
                gA = mpool.tile([P, TILES], dt, tag="gA")
                nc.vector.reduce_sum(gA[:], pB3, axis=X)
                nc.vector.affine_then_add(eta48n[:, 0:TILES], gA[:],
                                          X48[:, 0:TILES], -LR, 0.0)
                eta48 = eta48n
                if it < MAX_ITER - 1:
                    a2c = make_a2c(eta48)

            nc.gpsimd.dma_start(d_out[:], eta48[:])

    nc.finalize()
    _NC_CACHE["nc"] = nc
    return nc


# ---------------------------------------------------------------------------
# input staging (eta column order: A | t0 | k)
# ---------------------------------------------------------------------------

def _make_in_maps(ctc, aif, time, eta_nn, lambda_reg):
    f32 = np.float32
    import ml_dtypes
    bf16 = ml_dtypes.bfloat16

    tshc, G, GvT, nw, nwv, C_dc, creg = _preprocess(
        ctc, aif, time, eta_nn, lambda_reg)

    toc_v = 2.0 / C_dc
    sA, sK, sT0 = (1.0 - LR * creg).astype(np.float64)

    # argwbd[k, NC*t + c]: row t -> 1.0, row 16+t -> -tshc[c]
    # (kn carries +k; the minus sign lives here)
    argwbd = np.zeros((2 * TILES, TILES * NC), bf16)
    tshcf = tshc.astype(f32)
    for t_ in range(TILES):
        argwbd[t_, t_ * NC:(t_ + 1) * NC] = 1.0
        argwbd[TILES + t_, t_ * NC:(t_ + 1) * NC] = -tshcf
    # ggvd block-diag (16 tiles): [NC, 2*NC] blocks of [G | Gv^T]
    ggvd = np.zeros((TILES * NC, TILES * 2 * NC), bf16)
    blk = np.concatenate([G, GvT], axis=1)          # [NC, 2*NC]
    for tau in range(TILES):
        ggvd[tau * NC:(tau + 1) * NC, tau * 2 * NC:(tau + 1) * 2 * NC] = blk
    ident = np.eye(P, dtype=bf16)

    toc = np.full((P, TILES), toc_v, f32)
    s48 = np.zeros((P, 3 * TILES), f32)
    s48[:, 0:TILES] = sA          # A
    s48[:, TILES:2 * TILES] = sT0  # t0
    s48[:, 2 * TILES:] = sK        # k

    in_maps = []
    for m in range(N_CORES):
        rows = slice(m * ROWS_PER_CORE, (m + 1) * ROWS_PER_CORE)
        nwc = np.stack([nw[rows], nwv[rows]], axis=2)  # [16,128,2,NC]
        nwfull = np.ascontiguousarray(
            nwc.transpose(1, 0, 2, 3).reshape(P, TILES * 2 * NC)).astype(bf16)
        pr = eta_nn[0, :, rows, :].astype(np.float64)   # [3(A,k,t0), 16, 128]
        pr_atk = pr[[0, 2, 1]]                          # A | t0 | k
        eta0 = np.ascontiguousarray(
            pr_atk.transpose(2, 0, 1).reshape(P, 3 * TILES)).astype(f32)
        creg_atk = creg[[0, 2, 1]]
        cpl48 = np.zeros((P, 3 * TILES), f32)
        for c in range(3):
            cpl48[:, c * TILES:(c + 1) * TILES] = (
                LR * creg_atk[c] * pr_atk[c]).T
        ones16 = np.ones((P, TILES), f32)
        identf = np.eye(P, dtype=f32)
        fppack = np.concatenate([eta0, cpl48, s48, toc, ones16, identf],
                                axis=1)
        # ggvd is [128, 256] exactly (TILES*NC = 128)
        bfpack = np.concatenate([ggvd.astype(bf16), nwfull], axis=1)
        in_maps.append({
            "argwbd": argwbd, "bfpack": bfpack, "fppack": fppack,
        })
    return in_maps


def _emulate(in_maps):
    """Numpy replay of the device pipeline from staged arrays (debug aid)."""
    import ml_dtypes
    bf16 = ml_dtypes.bfloat16
    f32 = np.float32

    def bfq(x):
        return np.asarray(x, dtype=f32).astype(bf16).astype(f32)

    outs = []
    for mp in in_maps:
        argwbd = mp["argwbd"].astype(f32)
        bfpack = mp["bfpack"].astype(f32)
        fppack = mp["fppack"]
        ggvd = bfpack[:, 0:NT2]
        nwfull = bfpack[:, NT2:2 * NT2]
        eta48 = fppack[:, 0:3 * TILES].astype(f32).copy()
        cpl48 = fppack[:, 3 * TILES:6 * TILES]
        s48 = fppack[:, 6 * TILES:9 * TILES]
        toc = fppack[:, 9 * TILES:10 * TILES]
        for it in range(MAX_ITER):
            eA = eta48[:, 0:TILES]
            eT = eta48[:, TILES:2 * TILES]
            eK = eta48[:, 2 * TILES:]
            kn = np.zeros((P, 2 * TILES), f32)
            kn[:, 0:TILES] = eK * eT
            kn[:, TILES:] = eK
            knT = bfq(kn).T  # [32, 128] (bf16 at the knT copy)
            a2c = eA * toc
            X48 = eta48 * s48 + np.minimum(eta48, 0.0) * (-2.0 * LR) + cpl48
            arg2 = knT.T @ argwbd          # [128, 128]
            s1c = bfq(_sigmoid(arg2))
            sdc = bfq(s1c * (1.0 - s1c))
            argT = argwbd.T @ knT          # [128, 128]
            s1cT = bfq(_sigmoid(argT))
            yps = s1cT.T @ ggvd            # [128, 256]
            w1 = bfq(yps.reshape(P, TILES, 2 * NC)
                     * a2c[:, :, None]).reshape(P, -1)
            wg = bfq(w1 + nwfull)
            w4 = wg.reshape(P, TILES, 2, NC)
            pA = bfq(w4 * sdc.reshape(P, TILES, 1, NC))
            UV = pA.sum(-1)                 # [128, 16, 2]
            Um, Vm = UV[:, :, 0], UV[:, :, 1]
            h2 = Um * eK
            h1 = Um * eT - Vm
            gt0 = eA * h2
            gk = eA * h1
            pB = bfq(w4[:, :, 0, :] * s1c.reshape(P, TILES, NC))
            gA = pB.sum(-1)
            G48 = np.concatenate([gA, gt0, gk], axis=1)
            eta48 = X48 - LR * G48
        outs.append(eta48)
    out = np.zeros((1, 3, H, W), f32)
    for m, arr in enumerate(outs):
        rows = slice(m * ROWS_PER_CORE, (m + 1) * ROWS_PER_CORE)
        a3 = arr.reshape(P, 3, TILES)      # A | t0 | k
        out[0, 0, rows, :] = a3[:, 0, :].T
        out[0, 1, rows, :] = a3[:, 2, :].T
        out[0, 2, rows, :] = a3[:, 1, :].T
    return out


# ---------------------------------------------------------------------------
# public entry point
# ---------------------------------------------------------------------------

def kernel(ctc, aif, time, seg, eta_nn, lambda_reg):
    from concourse.bass_utils import run_bass_kernel_spmd

    ctc = np.asarray(ctc)
    aif = np.asarray(aif)
    time = np.asarray(time)
    eta_nn = np.asarray(eta_nn)
    lambda_reg = np.asarray(lambda_reg)

    in_maps = _make_in_maps(ctc, aif, time, eta_nn, lambda_reg)
    nc = _build_nc()
    res = run_bass_kernel_spmd(nc, in_maps, list(range(N_CORES)))

    out = np.zeros((1, 3, H, W), np.float32)
    for m in range(N_CORES):
        rows = slice(m * ROWS_PER_CORE, (m + 1) * ROWS_PER_CORE)
        arr = res.results[m]["out"]                  # [128, 48] A|t0|k
        a3 = arr.reshape(P, 3, TILES)
        out[0, 0, rows, :] = a3[:, 0, :].T
        out[0, 1, rows, :] = a3[:, 2, :].T
        out[0, 2, rows, :] = a3[:, 1, :].T
    return out
